# revision 39
# baseline (speedup 1.0000x reference)
"""Causal self-attention (nn_CausalSelfAttention) on 8 TRN2 NeuronCores.

Reference computation (B=2, T=2048, C=1024, H=16 heads, D=64):
    qkv = x @ W_attn.T + b_attn ; split q,k,v
    y   = softmax(causal(q k^T / sqrt(D))) v        (per head)
    out = y @ W_proj.T + b_proj

Sharding: batch (2-way) x head-group (4-way, 4 heads each) -> 8 cores.
Each core computes its batch's attention for its 4 heads plus the partial
c_proj contribution of those heads' channels; the host sums the 4 partials
per batch and adds b_proj once.

v2 layout (vs the v1 baseline): the exp stream on the Scalar engine is the
phase-3 bottleneck, and the Tensor engine total is the global one, so
everything else is moved off those two:
  - qk bias add: DVE tensor_scalar (was ACT Identity)
  - causal mask: DVE multiply with a precomputed lower-tri bf16 tile
    (was ~1us-per-call gpsimd affine_select on the exp->PV critical path)
  - projection output: DMA'd straight from PSUM (was ACT/DVE copy pass)
  - exp runs on [128,1024] two-bank PSUM tiles (halves the per-instruction
    ACT overhead); S^T diagonal tiles are computed full-width so the tile
    is always fully initialized before the single big exp
  - attention streams head-pairs; phase-1/phase-2/projection matmuls are
    interleaved one-per-round into the ACT-bound attention stream via a
    fill queue, so the in-order PE never idles waiting on exp
  - x is DMA'd in 512-column quarters and strips run forward so strip 0
    starts after ~1/4 of phase 1
"""
import math
from contextlib import ExitStack

import ml_dtypes
import numpy as np

import concourse.bacc as bacc
import concourse.bass as bass
import concourse.mybir as mybir
import concourse.tile as tile
from concourse.bass_utils import run_bass_kernel_spmd

F32 = mybir.dt.float32
BF16 = mybir.dt.bfloat16
MMDT = BF16                    # dtype for all TensorE-facing tensors

N_CORES = 8
B, T, C, H = 2, 2048, 1024, 16
D = 64
GROUPS = N_CORES // B          # head groups per batch = 4
HPC = H // GROUPS              # heads per core = 4
CS = HPC * D                   # channel slice per core = 256


def build_nc(T_=T, C_=C, CS_=CS):
    """Build + compile the per-core Bass program (SPMD: same program, 8 cores)."""
    TT = T_ // 128             # T tiles (16)
    KT = C_ // 128             # contraction tiles over C (8)
    NS = T_ // 512             # 512-wide query strips (4)
    HL = CS_ // D              # heads on this core (4)
    MQK = 2 * CS_ // 128       # m-tiles of the joint q|k channel block (4)
    KP = CS_ // 128            # contraction tiles for the projection (2)

    nc = bacc.Bacc("TRN2", target_bir_lowering=False, debug=False,
                   num_devices=N_CORES)

    # inputs are pre-relaid on the host so every DMA reads DRAM linearly:
    # xT quarter-major [q][p][kt][512], wqk m-slice-major [m][p][kt][128],
    # wv/wp partition-major [p][kt][n]
    xT = nc.dram_tensor("xT", [NS, 128, KT, 512], MMDT, kind="ExternalInput")
    wqkT = nc.dram_tensor("wqkT", [MQK, 128, KT, 128], MMDT,
                          kind="ExternalInput")
    bqk = nc.dram_tensor("bqk", [128, MQK], F32, kind="ExternalInput")
    wvT = nc.dram_tensor("wvT", [128, KT, CS_], MMDT, kind="ExternalInput")
    bv = nc.dram_tensor("bv", [1, HL * (D + 1)], F32, kind="ExternalInput")
    wpT = nc.dram_tensor("wpT", [128, KP, C_], MMDT, kind="ExternalInput")
    out = nc.dram_tensor("out", [T_, C_], MMDT, kind="ExternalOutput")

    scale = 1.0 / math.sqrt(D)
    ADD = mybir.AluOpType.add
    MUL = mybir.AluOpType.mult

    with tile.TileContext(nc) as tc, ExitStack() as ctx:
        px = ctx.enter_context(tc.tile_pool(name="px", bufs=1))
        pw = ctx.enter_context(tc.tile_pool(name="pw", bufs=1))
        pqk = ctx.enter_context(tc.tile_pool(name="pqk", bufs=1))
        pv = ctx.enter_context(tc.tile_pool(name="pv", bufs=1))
        py = ctx.enter_context(tc.tile_pool(name="py", bufs=1))
        ppt = ctx.enter_context(tc.tile_pool(name="ppt", bufs=12))
        pnorm = ctx.enter_context(tc.tile_pool(name="pnorm", bufs=4))
        pout = ctx.enter_context(tc.tile_pool(name="pout", bufs=4))
        # PSUM: pst 4x1-bank (S^T ring) + psy 2x1 (y accum) + pmm 2x1
        # (phase1/phase2/proj) = 8 banks
        pst = ctx.enter_context(tc.tile_pool(name="pst", bufs=4, space="PSUM"))
        psy = ctx.enter_context(tc.tile_pool(name="psy", bufs=2, space="PSUM"))
        pmm = ctx.enter_context(tc.tile_pool(name="pmm", bufs=2, space="PSUM"))

        # ---- input DMA ----
        # One dma_start per logical chunk, each a fully linear DRAM read.
        # One tile per chunk: a single strided DMA into a merged tile
        # coarsens the subtile dependency and consumers end up waiting for
        # everything — separate tiles give clean whole-tile deps.
        x_q = [px.tile([128, KT, 512], MMDT, tag=f"xq{q}", name=f"xq{q}")
               for q in range(NS)]
        wqk_m = [pw.tile([128, KT, 128], MMDT, tag=f"wqkm{m}", name=f"wqkm{m}")
                 for m in range(MQK)]
        nc.sync.dma_start(x_q[0][:], xT.ap()[0])
        for m in range(MQK):
            nc.sync.dma_start(wqk_m[m][:], wqkT.ap()[m])
        bqk_t = pw.tile([128, MQK], F32, tag="bqk", name="bqk_t")
        nc.sync.dma_start(bqk_t[:], bqk.ap())
        wv_t = pw.tile([128, KT, CS_], MMDT, tag="wv", name="wv_t")
        nc.sync.dma_start(wv_t[:], wvT.ap())
        bv_row = pw.tile([1, HL * (D + 1)], F32, tag="bv_row", name="bv_row")
        nc.sync.dma_start(bv_row[:], bv.ap())
        nc.sync.dma_start(x_q[1][:], xT.ap()[1])
        nc.sync.dma_start(x_q[2][:], xT.ap()[2])
        wp_t = pw.tile([128, KP, C_], MMDT, tag="wp", name="wp_t")
        nc.sync.dma_start(wp_t[:], wpT.ap())
        nc.sync.dma_start(x_q[3][:], xT.ap()[3])

        class XCols:
            """x_sb[k][:, a:b] view router over the quarter tiles."""

            def __init__(self, k):
                self.k = k

            def __getitem__(self, idx):
                _, cols = idx
                q, a = cols.start // 512, cols.start % 512
                assert cols.stop <= (q + 1) * 512, "x slice crosses quarters"
                return x_q[q][:, self.k, a:a + (cols.stop - cols.start)]

        x_sb = [XCols(k) for k in range(KT)]

        wv_sb = [wv_t[:, k, :] for k in range(KT)]
        wp_sb = [wp_t[:, k2, :] for k2 in range(KP)]
        bqk_sb = [bqk_t[:, m:m + 1] for m in range(MQK)]

        # PE warm-up: keep the array busy through the DMA head so the
        # p-state is at full clock when phase 1 lands
        warm = pw.tile([128, 512], MMDT, tag="warm", name="warm")
        nc.vector.memset(warm[:], 0.0)
        # 6 x ~790ns at the cold clock ends right as the first x/wqk land —
        # more would delay phase 1 behind the in-order PE queue
        for _ in range(6):
            wps = pst.tile([128, 512], F32, tag="st", name="warm_ps")
            nc.tensor.matmul(wps[:], warm[:, 0:128], warm[:],
                             start=True, stop=True)

        # bv is packed per head as [bias(D), 1.0]; the trailing 1.0 feeds the
        # ones column of v_aug (softmax denominator accumulator).
        bv_bc = pw.tile([128, HL * (D + 1)], F32, tag="bv_bc", name="bv_bc")
        nc.gpsimd.partition_broadcast(bv_bc[:], bv_row[:])

        # constant lower-triangular mask (key p kept for query f when f >= p)
        mask = pw.tile([128, 128], MMDT, tag="mask", name="mask")
        nc.gpsimd.memset(mask[:], 1.0)
        nc.gpsimd.affine_select(
            out=mask[:], in_=mask[:], compare_op=mybir.AluOpType.is_ge,
            fill=0.0, base=0, pattern=[[1, 128]], channel_multiplier=-1)

        qk_sb = [pqk.tile([128, T_], MMDT, tag=f"qk{m}", name=f"qk{m}")
                 for m in range(MQK)]
        v_sb = [pv.tile([128, HL * (D + 1)], MMDT, tag=f"v{t}", name=f"v{t}")
                for t in range(TT)]
        y_sb = [py.tile([128, T_], MMDT, tag=f"y{k2}", name=f"y{k2}")
                for k2 in range(KP)]

        # ---- phase 1 / phase 2 / projection emitters (fill work) ----
        def emit_p1(s, m):
            ps = pmm.tile([128, 512], F32, tag="mm", name="ps_qk")
            cl = slice(s * 512, (s + 1) * 512)
            for k in range(KT):
                nc.tensor.matmul(
                    ps[:], wqk_m[m][:, k, :],
                    x_sb[k][:, cl], start=(k == 0), stop=(k == KT - 1))
            nc.vector.tensor_scalar(
                qk_sb[m][:, cl], ps[:], bqk_sb[m][:], None, op0=ADD)

        def emit_v(t):
            ps = pmm.tile([128, CS_], F32, tag="mm", name="ps_v")
            for k in range(KT):
                nc.tensor.matmul(
                    ps[:], x_sb[k][:, t * 128:(t + 1) * 128], wv_sb[k][:],
                    start=(k == 0), stop=(k == KT - 1))
            vgrp = v_sb[t][:].rearrange("p (g e) -> p g e", e=D + 1)
            vsrc = ps[:].rearrange("p (g e) -> p g e", e=D)
            bgrp = bv_bc[:].rearrange("p (g e) -> p g e", e=D + 1)
            nc.vector.tensor_tensor(vgrp[:, :, 0:D], vsrc, bgrp[:, :, 0:D],
                                    op=ADD)
            nc.vector.tensor_copy(vgrp[:, :, D:D + 1], bgrp[:, :, D:D + 1])

        ot_tiles = {}

        def emit_proj(t, cc, pool=None, tag="mm"):
            ps = (pool or pmm).tile([128, 512], F32, tag=tag, name="ps_o")
            cl = slice(cc * 512, (cc + 1) * 512)
            for k2 in range(KP):
                nc.tensor.matmul(
                    ps[:], y_sb[k2][:, t * 128:(t + 1) * 128],
                    wp_sb[k2][:, cl], start=(k2 == 0), stop=(k2 == KP - 1))
            if cc == 0:
                ot_tiles[t] = pout.tile([128, C_], MMDT, tag="ot", name="ot")
            # stage on DVE (ACT is the exp engine; keep it clean) and ship
            # one [128, C] DMA per t-tile instead of one per half
            nc.vector.tensor_copy(ot_tiles[t][:, cl], ps[:])
            if cc == C_ // 512 - 1:
                # tail drain: alternate queues so the last 8 output DMAs
                # aren't serialized behind one queue's 565ns issues (ACT is
                # idle once its last exp retires)
                eng = nc.scalar if t >= 12 and t % 2 else nc.sync
                eng.dma_start(out.ap()[t * 128:(t + 1) * 128, :],
                              ot_tiles.pop(t)[:])

        fill = []

        def pump():
            if fill:
                fill.pop(0)()

        # ---- phase 1 strip 0 + v tiles 0-3 up front ----
        for m in range(MQK):
            emit_p1(0, m)
        for t in range(4):
            emit_v(t)
        # rest of phase 1/2 is pumped into the attention stream
        for s in range(1, NS):
            for m in range(MQK):
                fill.append(lambda s=s, m=m: emit_p1(s, m))
            for t in range(4 * s, 4 * s + 4):
                fill.append(lambda t=t: emit_v(t))

        def head_slices(hl):
            lo = (hl % 2) * D
            qh = qk_sb[hl // 2][lo:lo + D, :]
            kh = qk_sb[KP + hl // 2][lo:lo + D, :]
            return qh, kh

        # ---- phase 3: attention, head-pair streaming, forward strips ----
        for s in range(NS):
            nt = 4 * s + 4
            ql = slice(s * 512, (s + 1) * 512)
            for pair in range(HL // 2):
                heads = (2 * pair, 2 * pair + 1)
                yps = [psy.tile([D + 1, 512], F32, tag="yp", name=f"yp{hl}")
                       for hl in heads]
                # software-pipelined issue: S^T(n+1) goes to the in-order PE
                # queue before PV(n), and a fill task slots between them, so
                # the PE never head-blocks on exp(n)
                pts = [None] * nt

                def emit_st(n):
                    off = max(0, (n - 4 * s)) * 128
                    pair_pt = []
                    for u, hl in enumerate(heads):
                        qh, kh = head_slices(hl)
                        st = pst.tile([128, 512], F32, tag="st", name="st")
                        nc.tensor.matmul(
                            st[:, off:512],
                            kh[:, n * 128:(n + 1) * 128],
                            qh[:, s * 512 + off:(s + 1) * 512],
                            start=True, stop=True)
                        pt = ppt.tile([128, 512], MMDT, tag="pt",
                                      name="ptile")
                        nc.scalar.activation(
                            pt[:, off:512], st[:, off:512],
                            mybir.ActivationFunctionType.Exp, scale=scale)
                        if n >= 4 * s:
                            # mixed diagonal block (including off == 0):
                            # zero the strict upper triangle (key > query)
                            nc.vector.tensor_tensor(
                                pt[:, off:off + 128], pt[:, off:off + 128],
                                mask[:], op=MUL)
                        pair_pt.append(pt)
                    pts[n] = pair_pt

                emit_st(0)
                if nt > 1:
                    emit_st(1)
                for n in range(nt):
                    off = max(0, (n - 4 * s)) * 128
                    if n + 2 < nt:
                        emit_st(n + 2)
                    pump()
                    for u, hl in enumerate(heads):
                        nc.tensor.matmul(
                            yps[u][:, off:512],
                            v_sb[n][:, hl * (D + 1):(hl + 1) * (D + 1)],
                            pts[n][u][:, off:512],
                            start=(n == 0), stop=(n == nt - 1))
                for u, hl in enumerate(heads):
                    lo = (hl % 2) * D
                    # drain the whole accumulator to SBUF in ONE copy so the
                    # PSUM bank frees ~3us earlier (the next pair's first PV
                    # reuses it); the rest of the chain runs off SBUF.
                    # (The copy is also required: the custom-DVE reciprocal
                    # misreads PSUM, and the mult may read only one PSUM
                    # operand.)
                    yc = pnorm.tile([D + 1, 512], F32, tag="yc", name="yc")
                    nc.vector.tensor_copy(yc[:], yps[u][:])
                    # custom-DVE ops misread at nonzero base partition:
                    # bounce the denominator row to a partition-0 tile first
                    rs = pnorm.tile([1, 512], F32, tag="rs", name="rs")
                    nc.vector.tensor_copy(rs[:], yc[D:D + 1, :])
                    rr = pnorm.tile([1, 512], F32, tag="rr", name="rr")
                    nc.vector.reciprocal_approx_fast(rr[:], rs[:])
                    rb = pnorm.tile([D, 512], F32, tag="rb", name="rb")
                    nc.gpsimd.partition_broadcast(rb[:], rr[:])
                    nc.vector.tensor_tensor(
                        y_sb[(hl * D) // 128][lo:lo + D, ql],
                        yc[0:D, :], rb[:], op=MUL)
            # this strip's projection joins the fill queue (runs during the
            # next strip's rounds); the last strip's drains below across the
            # now-idle pst/psy slots so the tail isn't gated on two pmm banks
            last = s == NS - 1
            drain_pools = [(pmm, "mm"), (pst, "st"), (psy, "yp")]
            for i, (t, cc) in enumerate(
                    (t, cc) for t in range(4 * s, 4 * s + 4)
                    for cc in range(C_ // 512)):
                if last:
                    pool, tag = drain_pools[i % 3]
                    emit_proj(t, cc, pool=pool, tag=tag)
                else:
                    fill.append(lambda t=t, cc=cc: emit_proj(t, cc))
        assert not fill, f"{len(fill)} fill tasks never pumped"

    nc.compile()
    return nc


def make_in_maps(x, W_attn, b_attn, W_proj):
    """Shard full inputs into the 8 per-core input dicts."""
    x = np.asarray(x, dtype=np.float32)
    W_attn = np.asarray(W_attn, dtype=np.float32)
    b_attn = np.asarray(b_attn, dtype=np.float32)
    W_proj = np.asarray(W_proj, dtype=np.float32)
    Cq = C
    in_maps = []
    xTb = [np.ascontiguousarray(x[b_].T) for b_ in range(B)]
    for core in range(N_CORES):
        b_ = core // GROUPS
        g = core % GROUPS
        sl = slice(g * CS, (g + 1) * CS)
        wq = W_attn[sl, :]
        wk = W_attn[Cq + g * CS:Cq + (g + 1) * CS, :]
        wv = W_attn[2 * Cq + g * CS:2 * Cq + (g + 1) * CS, :]
        bq = b_attn[sl]
        bk = b_attn[Cq + g * CS:Cq + (g + 1) * CS]
        bvs = b_attn[2 * Cq + g * CS:2 * Cq + (g + 1) * CS]
        # DRAM layouts matched to the kernel's linear DMAs:
        #   xT   [NS, 128, KT, 512]: quarter q, partition p, k-tile, col
        #   wqkT [MQK, 128, KT, 128]: m-slice, partition, k-tile, col
        #   wvT  [128, KT, CS] / wpT [128, KP, C]: partition, k-tile, col
        xq = (xTb[b_].reshape(C // 128, 128, T // 512, 512)
              .transpose(2, 1, 0, 3))
        wqkT = (np.concatenate([wq, wk], 0).T
                .reshape(C // 128, 128, 2 * CS // 128, 128)
                .transpose(2, 1, 0, 3))
        wvT = wv.T.reshape(C // 128, 128, CS).transpose(1, 0, 2)
        wpT = (W_proj[:, g * CS:(g + 1) * CS].T
               .reshape(CS // 128, 128, C).transpose(1, 0, 2))
        in_maps.append({
            "xT": np.ascontiguousarray(xq).astype(ml_dtypes.bfloat16),
            "wqkT": np.ascontiguousarray(wqkT).astype(ml_dtypes.bfloat16),
            "bqk": np.ascontiguousarray(
                np.concatenate([bq, bk]).reshape(2 * CS // 128, 128).T),
            "wvT": np.ascontiguousarray(wvT).astype(ml_dtypes.bfloat16),
            "bv": np.ascontiguousarray(
                np.concatenate([bvs.reshape(HPC, D),
                                np.ones((HPC, 1), np.float32)],
                               axis=1).reshape(1, HPC * (D + 1))),
            "wpT": np.ascontiguousarray(wpT).astype(ml_dtypes.bfloat16),
        })
    return in_maps


_NC = None


def _get_nc():
    global _NC
    if _NC is None:
        _NC = build_nc()
    return _NC


def run(x, W_attn, b_attn, W_proj, b_proj, trace=False):
    nc = _get_nc()
    in_maps = make_in_maps(x, W_attn, b_attn, W_proj)
    res = run_bass_kernel_spmd(nc, in_maps, core_ids=list(range(N_CORES)),
                               trace=trace)
    out = np.zeros((B, T, C), dtype=np.float32)
    for core in range(N_CORES):
        out[core // GROUPS] += res.results[core]["out"].astype(np.float32)
    out += np.asarray(b_proj, dtype=np.float32)[None, None, :]
    return out, res


def kernel(x, W_attn, b_attn, W_proj, b_proj):
    out, _ = run(x, W_attn, b_attn, W_proj, b_proj, trace=False)
    return out


# revision 40
# speedup vs baseline: 1.0383x; 1.0383x over previous
"""Causal self-attention (nn_CausalSelfAttention) on 8 TRN2 NeuronCores.

Reference computation (B=2, T=2048, C=1024, H=16 heads, D=64):
    qkv = x @ W_attn.T + b_attn ; split q,k,v
    y   = softmax(causal(q k^T / sqrt(D))) v        (per head)
    out = y @ W_proj.T + b_proj

Sharding: batch (2-way) x head-group (4-way, 4 heads each) -> 8 cores.
Each core computes its batch's attention for its 4 heads plus the partial
c_proj contribution of those heads' channels; the host sums the 4 partials
per batch and adds b_proj once.

v2 layout (vs the v1 baseline): the exp stream on the Scalar engine is the
phase-3 bottleneck, and the Tensor engine total is the global one, so
everything else is moved off those two:
  - qk bias add: DVE tensor_scalar (was ACT Identity)
  - causal mask: DVE multiply with a precomputed lower-tri bf16 tile
    (was ~1us-per-call gpsimd affine_select on the exp->PV critical path)
  - projection output: DMA'd straight from PSUM (was ACT/DVE copy pass)
  - exp runs on [128,1024] two-bank PSUM tiles (halves the per-instruction
    ACT overhead); S^T diagonal tiles are computed full-width so the tile
    is always fully initialized before the single big exp
  - attention streams head-pairs; phase-1/phase-2/projection matmuls are
    interleaved one-per-round into the ACT-bound attention stream via a
    fill queue, so the in-order PE never idles waiting on exp
  - x is DMA'd in 512-column quarters and strips run forward so strip 0
    starts after ~1/4 of phase 1
"""
import math
from contextlib import ExitStack

import ml_dtypes
import numpy as np

import concourse.bacc as bacc
import concourse.bass as bass
import concourse.mybir as mybir
import concourse.tile as tile
from concourse.bass_utils import run_bass_kernel_spmd

F32 = mybir.dt.float32
BF16 = mybir.dt.bfloat16
MMDT = BF16                    # dtype for all TensorE-facing tensors

N_CORES = 8
B, T, C, H = 2, 2048, 1024, 16
D = 64
GROUPS = N_CORES // B          # head groups per batch = 4
HPC = H // GROUPS              # heads per core = 4
CS = HPC * D                   # channel slice per core = 256


def build_nc(T_=T, C_=C, CS_=CS):
    """Build + compile the per-core Bass program (SPMD: same program, 8 cores)."""
    TT = T_ // 128             # T tiles (16)
    KT = C_ // 128             # contraction tiles over C (8)
    NS = T_ // 512             # 512-wide query strips (4)
    HL = CS_ // D              # heads on this core (4)
    MQK = 2 * CS_ // 128       # m-tiles of the joint q|k channel block (4)
    KP = CS_ // 128            # contraction tiles for the projection (2)

    nc = bacc.Bacc("TRN2", target_bir_lowering=False, debug=False,
                   num_devices=N_CORES)

    # inputs are pre-relaid on the host so every DMA reads DRAM linearly:
    # xT quarter-major [q][p][kt][512], wqk m-slice-major [m][p][kt][128],
    # wv/wp partition-major [p][kt][n]
    xT = nc.dram_tensor("xT", [NS, 128, KT, 512], MMDT, kind="ExternalInput")
    wqkT = nc.dram_tensor("wqkT", [MQK, 128, KT, 128], MMDT,
                          kind="ExternalInput")
    bqk = nc.dram_tensor("bqk", [128, MQK], F32, kind="ExternalInput")
    wvT = nc.dram_tensor("wvT", [128, KT, CS_], MMDT, kind="ExternalInput")
    bv = nc.dram_tensor("bv", [1, HL * (D + 1)], F32, kind="ExternalInput")
    wpT = nc.dram_tensor("wpT", [128, KP, C_], MMDT, kind="ExternalInput")
    out = nc.dram_tensor("out", [T_, C_], MMDT, kind="ExternalOutput")

    scale = 1.0 / math.sqrt(D)
    ADD = mybir.AluOpType.add
    MUL = mybir.AluOpType.mult

    with tile.TileContext(nc) as tc, ExitStack() as ctx:
        px = ctx.enter_context(tc.tile_pool(name="px", bufs=1))
        pw = ctx.enter_context(tc.tile_pool(name="pw", bufs=1))
        pqk = ctx.enter_context(tc.tile_pool(name="pqk", bufs=1))
        pv = ctx.enter_context(tc.tile_pool(name="pv", bufs=1))
        py = ctx.enter_context(tc.tile_pool(name="py", bufs=1))
        ppt = ctx.enter_context(tc.tile_pool(name="ppt", bufs=12))
        pnorm = ctx.enter_context(tc.tile_pool(name="pnorm", bufs=4))
        pout = ctx.enter_context(tc.tile_pool(name="pout", bufs=4))
        # PSUM: pst 4x1-bank (S^T ring) + psy 2x1 (y accum) + pmm 2x1
        # (phase1/phase2/proj) = 8 banks
        pst = ctx.enter_context(tc.tile_pool(name="pst", bufs=4, space="PSUM"))
        psy = ctx.enter_context(tc.tile_pool(name="psy", bufs=2, space="PSUM"))
        pmm = ctx.enter_context(tc.tile_pool(name="pmm", bufs=2, space="PSUM"))

        # ---- input DMA ----
        # One dma_start per logical chunk, each a fully linear DRAM read.
        # One tile per chunk: a single strided DMA into a merged tile
        # coarsens the subtile dependency and consumers end up waiting for
        # everything — separate tiles give clean whole-tile deps.
        x_q = [px.tile([128, KT, 512], MMDT, tag=f"xq{q}", name=f"xq{q}")
               for q in range(NS)]
        wqk_m = [pw.tile([128, KT, 128], MMDT, tag=f"wqkm{m}", name=f"wqkm{m}")
                 for m in range(MQK)]
        nc.sync.dma_start(x_q[0][:], xT.ap()[0])
        for m in range(MQK):
            nc.sync.dma_start(wqk_m[m][:], wqkT.ap()[m])
        bqk_t = pw.tile([128, MQK], F32, tag="bqk", name="bqk_t")
        nc.sync.dma_start(bqk_t[:], bqk.ap())
        wv_t = pw.tile([128, KT, CS_], MMDT, tag="wv", name="wv_t")
        nc.sync.dma_start(wv_t[:], wvT.ap())
        bv_row = pw.tile([1, HL * (D + 1)], F32, tag="bv_row", name="bv_row")
        nc.sync.dma_start(bv_row[:], bv.ap())
        nc.sync.dma_start(x_q[1][:], xT.ap()[1])
        nc.sync.dma_start(x_q[2][:], xT.ap()[2])
        wp_t = pw.tile([128, KP, C_], MMDT, tag="wp", name="wp_t")
        nc.sync.dma_start(wp_t[:], wpT.ap())
        nc.sync.dma_start(x_q[3][:], xT.ap()[3])

        class XCols:
            """x_sb[k][:, a:b] view router over the quarter tiles."""

            def __init__(self, k):
                self.k = k

            def __getitem__(self, idx):
                _, cols = idx
                q, a = cols.start // 512, cols.start % 512
                assert cols.stop <= (q + 1) * 512, "x slice crosses quarters"
                return x_q[q][:, self.k, a:a + (cols.stop - cols.start)]

        x_sb = [XCols(k) for k in range(KT)]

        wv_sb = [wv_t[:, k, :] for k in range(KT)]
        wp_sb = [wp_t[:, k2, :] for k2 in range(KP)]
        bqk_sb = [bqk_t[:, m:m + 1] for m in range(MQK)]

        # PE warm-up: keep the array busy through the DMA head so the
        # p-state is at full clock when phase 1 lands
        warm = pw.tile([128, 512], MMDT, tag="warm", name="warm")
        nc.vector.memset(warm[:], 0.0)
        # 6 x ~790ns at the cold clock ends right as the first x/wqk land —
        # more would delay phase 1 behind the in-order PE queue
        for _ in range(6):
            wps = pst.tile([128, 512], F32, tag="st", name="warm_ps")
            nc.tensor.matmul(wps[:], warm[:, 0:128], warm[:],
                             start=True, stop=True)

        # bv is packed per head as [bias(D), 1.0]; the trailing 1.0 feeds the
        # ones column of v_aug (softmax denominator accumulator).
        bv_bc = pw.tile([128, HL * (D + 1)], F32, tag="bv_bc", name="bv_bc")
        nc.gpsimd.partition_broadcast(bv_bc[:], bv_row[:])

        # constant lower-triangular mask (key p kept for query f when f >= p)
        mask = pw.tile([128, 128], MMDT, tag="mask", name="mask")
        nc.gpsimd.memset(mask[:], 1.0)
        nc.gpsimd.affine_select(
            out=mask[:], in_=mask[:], compare_op=mybir.AluOpType.is_ge,
            fill=0.0, base=0, pattern=[[1, 128]], channel_multiplier=-1)

        qk_sb = [pqk.tile([128, T_], MMDT, tag=f"qk{m}", name=f"qk{m}")
                 for m in range(MQK)]
        v_sb = [pv.tile([128, HL * (D + 1)], MMDT, tag=f"v{t}", name=f"v{t}")
                for t in range(TT)]
        y_sb = [py.tile([128, T_], MMDT, tag=f"y{k2}", name=f"y{k2}")
                for k2 in range(KP)]

        # ---- phase 1 / phase 2 / projection emitters (fill work) ----
        def emit_p1(s, m):
            ps = pmm.tile([128, 512], F32, tag="mm", name="ps_qk")
            cl = slice(s * 512, (s + 1) * 512)
            for k in range(KT):
                nc.tensor.matmul(
                    ps[:], wqk_m[m][:, k, :],
                    x_sb[k][:, cl], start=(k == 0), stop=(k == KT - 1))
            nc.vector.tensor_scalar(
                qk_sb[m][:, cl], ps[:], bqk_sb[m][:], None, op0=ADD)

        def emit_v(t):
            ps = pmm.tile([128, CS_], F32, tag="mm", name="ps_v")
            for k in range(KT):
                nc.tensor.matmul(
                    ps[:], x_sb[k][:, t * 128:(t + 1) * 128], wv_sb[k][:],
                    start=(k == 0), stop=(k == KT - 1))
            vgrp = v_sb[t][:].rearrange("p (g e) -> p g e", e=D + 1)
            vsrc = ps[:].rearrange("p (g e) -> p g e", e=D)
            bgrp = bv_bc[:].rearrange("p (g e) -> p g e", e=D + 1)
            nc.vector.tensor_tensor(vgrp[:, :, 0:D], vsrc, bgrp[:, :, 0:D],
                                    op=ADD)
            nc.vector.tensor_copy(vgrp[:, :, D:D + 1], bgrp[:, :, D:D + 1])

        ot_tiles = {}

        def emit_proj(t, cc, pool=None, tag="mm"):
            ps = (pool or pmm).tile([128, 512], F32, tag=tag, name="ps_o")
            cl = slice(cc * 512, (cc + 1) * 512)
            for k2 in range(KP):
                nc.tensor.matmul(
                    ps[:], y_sb[k2][:, t * 128:(t + 1) * 128],
                    wp_sb[k2][:, cl], start=(k2 == 0), stop=(k2 == KP - 1))
            if cc == 0:
                ot_tiles[t] = pout.tile([128, C_], MMDT, tag="ot", name="ot")
            # stage on DVE (ACT is the exp engine; keep it clean) and ship
            # one [128, C] DMA per t-tile instead of one per half
            nc.vector.tensor_copy(ot_tiles[t][:, cl], ps[:])
            if cc == C_ // 512 - 1:
                # tail drain: alternate queues so the last 8 output DMAs
                # aren't serialized behind one queue's 565ns issues (ACT is
                # idle once its last exp retires)
                eng = nc.scalar if t >= 12 and t % 2 else nc.sync
                eng.dma_start(out.ap()[t * 128:(t + 1) * 128, :],
                              ot_tiles.pop(t)[:])

        fill = []

        def pump():
            if fill:
                fill.pop(0)()

        # ---- phase 1 strip 0 + v tiles 0-3 up front ----
        for m in range(MQK):
            emit_p1(0, m)
        for t in range(4):
            emit_v(t)
        # rest of phase 1/2 is pumped into the attention stream
        for s in range(1, NS):
            for m in range(MQK):
                fill.append(lambda s=s, m=m: emit_p1(s, m))
            for t in range(4 * s, 4 * s + 4):
                fill.append(lambda t=t: emit_v(t))

        def head_slices(hl):
            lo = (hl % 2) * D
            qh = qk_sb[hl // 2][lo:lo + D, :]
            kh = qk_sb[KP + hl // 2][lo:lo + D, :]
            return qh, kh

        # ---- phase 3: attention, head-pair streaming, forward strips ----
        for s in range(NS):
            nt = 4 * s + 4
            ql = slice(s * 512, (s + 1) * 512)
            for pair in range(HL // 2):
                heads = (2 * pair, 2 * pair + 1)
                yps = [psy.tile([D + 1, 512], F32, tag="yp", name=f"yp{hl}")
                       for hl in heads]
                # software-pipelined issue: S^T(n+1) goes to the in-order PE
                # queue before PV(n), and a fill task slots between them, so
                # the PE never head-blocks on exp(n)
                pts = [None] * nt

                def emit_st(n):
                    off = max(0, (n - 4 * s)) * 128
                    pair_pt = []
                    for u, hl in enumerate(heads):
                        qh, kh = head_slices(hl)
                        st = pst.tile([128, 512], F32, tag="st", name="st")
                        nc.tensor.matmul(
                            st[:, off:512],
                            kh[:, n * 128:(n + 1) * 128],
                            qh[:, s * 512 + off:(s + 1) * 512],
                            start=True, stop=True)
                        pt = ppt.tile([128, 512], MMDT, tag="pt",
                                      name="ptile")
                        nc.scalar.activation(
                            pt[:, off:512], st[:, off:512],
                            mybir.ActivationFunctionType.Exp, scale=scale)
                        if n >= 4 * s:
                            # mixed diagonal block (including off == 0):
                            # zero the strict upper triangle (key > query)
                            nc.vector.tensor_tensor(
                                pt[:, off:off + 128], pt[:, off:off + 128],
                                mask[:], op=MUL)
                        pair_pt.append(pt)
                    pts[n] = pair_pt

                emit_st(0)
                for n in range(nt):
                    off = max(0, (n - 4 * s)) * 128
                    if n + 1 < nt:
                        emit_st(n + 1)
                    pump()
                    for u, hl in enumerate(heads):
                        nc.tensor.matmul(
                            yps[u][:, off:512],
                            v_sb[n][:, hl * (D + 1):(hl + 1) * (D + 1)],
                            pts[n][u][:, off:512],
                            start=(n == 0), stop=(n == nt - 1))
                for u, hl in enumerate(heads):
                    lo = (hl % 2) * D
                    # drain the whole accumulator to SBUF in ONE copy so the
                    # PSUM bank frees ~3us earlier (the next pair's first PV
                    # reuses it); the rest of the chain runs off SBUF.
                    # (The copy is also required: the custom-DVE reciprocal
                    # misreads PSUM, and the mult may read only one PSUM
                    # operand.)
                    yc = pnorm.tile([D + 1, 512], F32, tag="yc", name="yc")
                    nc.vector.tensor_copy(yc[:], yps[u][:])
                    # custom-DVE ops misread at nonzero base partition:
                    # bounce the denominator row to a partition-0 tile first
                    rs = pnorm.tile([1, 512], F32, tag="rs", name="rs")
                    nc.vector.tensor_copy(rs[:], yc[D:D + 1, :])
                    rr = pnorm.tile([1, 512], F32, tag="rr", name="rr")
                    nc.vector.reciprocal_approx_fast(rr[:], rs[:])
                    rb = pnorm.tile([D, 512], F32, tag="rb", name="rb")
                    nc.gpsimd.partition_broadcast(rb[:], rr[:])
                    nc.vector.tensor_tensor(
                        y_sb[(hl * D) // 128][lo:lo + D, ql],
                        yc[0:D, :], rb[:], op=MUL)
            # this strip's projection joins the fill queue (runs during the
            # next strip's rounds); the last strip's drains below across the
            # now-idle pst/psy slots so the tail isn't gated on two pmm banks
            last = s == NS - 1
            drain_pools = [(pmm, "mm"), (pst, "st"), (psy, "yp")]
            for i, (t, cc) in enumerate(
                    (t, cc) for t in range(4 * s, 4 * s + 4)
                    for cc in range(C_ // 512)):
                if last:
                    pool, tag = drain_pools[i % 3]
                    emit_proj(t, cc, pool=pool, tag=tag)
                else:
                    fill.append(lambda t=t, cc=cc: emit_proj(t, cc))
        assert not fill, f"{len(fill)} fill tasks never pumped"

    nc.compile()
    return nc


def make_in_maps(x, W_attn, b_attn, W_proj):
    """Shard full inputs into the 8 per-core input dicts."""
    x = np.asarray(x, dtype=np.float32)
    W_attn = np.asarray(W_attn, dtype=np.float32)
    b_attn = np.asarray(b_attn, dtype=np.float32)
    W_proj = np.asarray(W_proj, dtype=np.float32)
    Cq = C
    in_maps = []
    xTb = [np.ascontiguousarray(x[b_].T) for b_ in range(B)]
    for core in range(N_CORES):
        b_ = core // GROUPS
        g = core % GROUPS
        sl = slice(g * CS, (g + 1) * CS)
        wq = W_attn[sl, :]
        wk = W_attn[Cq + g * CS:Cq + (g + 1) * CS, :]
        wv = W_attn[2 * Cq + g * CS:2 * Cq + (g + 1) * CS, :]
        bq = b_attn[sl]
        bk = b_attn[Cq + g * CS:Cq + (g + 1) * CS]
        bvs = b_attn[2 * Cq + g * CS:2 * Cq + (g + 1) * CS]
        # DRAM layouts matched to the kernel's linear DMAs:
        #   xT   [NS, 128, KT, 512]: quarter q, partition p, k-tile, col
        #   wqkT [MQK, 128, KT, 128]: m-slice, partition, k-tile, col
        #   wvT  [128, KT, CS] / wpT [128, KP, C]: partition, k-tile, col
        xq = (xTb[b_].reshape(C // 128, 128, T // 512, 512)
              .transpose(2, 1, 0, 3))
        wqkT = (np.concatenate([wq, wk], 0).T
                .reshape(C // 128, 128, 2 * CS // 128, 128)
                .transpose(2, 1, 0, 3))
        wvT = wv.T.reshape(C // 128, 128, CS).transpose(1, 0, 2)
        wpT = (W_proj[:, g * CS:(g + 1) * CS].T
               .reshape(CS // 128, 128, C).transpose(1, 0, 2))
        in_maps.append({
            "xT": np.ascontiguousarray(xq).astype(ml_dtypes.bfloat16),
            "wqkT": np.ascontiguousarray(wqkT).astype(ml_dtypes.bfloat16),
            "bqk": np.ascontiguousarray(
                np.concatenate([bq, bk]).reshape(2 * CS // 128, 128).T),
            "wvT": np.ascontiguousarray(wvT).astype(ml_dtypes.bfloat16),
            "bv": np.ascontiguousarray(
                np.concatenate([bvs.reshape(HPC, D),
                                np.ones((HPC, 1), np.float32)],
                               axis=1).reshape(1, HPC * (D + 1))),
            "wpT": np.ascontiguousarray(wpT).astype(ml_dtypes.bfloat16),
        })
    return in_maps


_NC = None


def _get_nc():
    global _NC
    if _NC is None:
        _NC = build_nc()
    return _NC


def run(x, W_attn, b_attn, W_proj, b_proj, trace=False):
    nc = _get_nc()
    in_maps = make_in_maps(x, W_attn, b_attn, W_proj)
    res = run_bass_kernel_spmd(nc, in_maps, core_ids=list(range(N_CORES)),
                               trace=trace)
    out = np.zeros((B, T, C), dtype=np.float32)
    for core in range(N_CORES):
        out[core // GROUPS] += res.results[core]["out"].astype(np.float32)
    out += np.asarray(b_proj, dtype=np.float32)[None, None, :]
    return out, res


def kernel(x, W_attn, b_attn, W_proj, b_proj):
    out, _ = run(x, W_attn, b_attn, W_proj, b_proj, trace=False)
    return out
